# revision 13
# baseline (speedup 1.0000x reference)
"""Trainium2 Bass kernel for nn_AttLSTM (attention-LSTM, K=4 steps).

Math per step (reference):
    a = softmax(h @ g_S.T, axis=1)            # [B, S]
    r = a @ g_S                               # [B, D]
    gates = f_x @ W_ih.T + b_ih + [h, r] @ W_hh.T + b_hh
    i, f, g, o = split(gates, 4)
    c' = sig(f)*c + sig(i)*tanh(g); h' = sig(o)*tanh(c') + f_x

Design (per core, data-parallel over batch: B_loc = 512 rows/core):
  - fp16 matmul operands everywhere, f32 PSUM accumulation.
  - x @ W_ih.T + biases precomputed once (x == f_x every step) -> xw.
  - g_S kept two ways: transposed [D, S] resident in SBUF fp16 (g_T, rhs of
    the logits matmul) and natural [S, D] streamed per step from a DRAM fp16
    scratch copy (rhs of the readout matmul).
  - ALL transposes via PE transpose-mode in groups of 4-8 [128,128] blocks
    into one fp16 PSUM bank + one (strided) copy back to SBUF. DMA xbar
    transposes serialize at ~1.5us each on HWDGE - never use them here.
  - softmax per 128-row b-tile: per-512-chunk negated max (DVE, from PSUM),
    exp with per-chunk bias straight from PSUM (ACT) + accum_out row-sums,
    then a global per-row rescale p *= exp(m_chunk - m_row) before use.
  - sigmoid computed as 0.5*tanh(x/2)+0.5 so the single `exp_and_others`
    ACT table set (Exp + Tanh) serves the whole kernel.
  - LSTM pointwise math as fused scalar_tensor_tensor ops on DVE, carrying
    z = 2c as state.
  - b-tile schedule A(0) A(1) B(0) A(2) B(1) A(3) B(2) B(3) keeps PE dense
    while only two [128, S] probability buffers are live.
"""

import os
import sys

import numpy as np

for _p in ("/opt/trn_rl_repo",):
    if _p not in sys.path and os.path.isdir(_p):
        sys.path.insert(0, _p)

# Problem sizes (hardcoded per spec).
B, S, D = 4096, 8192, 512
H = D
N_CORES = 8
B_LOC = B // N_CORES          # 512 rows per core
K_STEPS = 4
P = 128                       # partitions


def build_bass(b_loc=B_LOC, s=S, k_steps=K_STEPS):
    import concourse.mybir as mybir
    import concourse.tile as tile
    from concourse import bacc
    from concourse.masks import make_identity
    from contextlib import ExitStack

    f32 = mybir.dt.float32
    f16 = mybir.dt.float16
    AF = mybir.ActivationFunctionType
    ALU = mybir.AluOpType
    AX = mybir.AxisListType

    nb = b_loc // P               # b-tiles per core
    nd = D // P                   # contraction chunks over D
    ns = s // 512                 # s-chunks of 512
    nt = s // P                   # s-tiles of 128
    ng = (4 * H) // 512           # gate chunks

    nc = bacc.Bacc("TRN2", target_bir_lowering=False, debug=False)

    f_x = nc.dram_tensor("f_x", [b_loc, D], f32, kind="ExternalInput")
    g_S = nc.dram_tensor("g_S", [s, D], f32, kind="ExternalInput")
    W_ih = nc.dram_tensor("W_ih", [4 * H, D], f32, kind="ExternalInput")
    W_hh = nc.dram_tensor("W_hh", [4 * H, 2 * H], f32, kind="ExternalInput")
    b_ih = nc.dram_tensor("b_ih", [4 * H], f32, kind="ExternalInput")
    b_hh = nc.dram_tensor("b_hh", [4 * H], f32, kind="ExternalInput")
    out = nc.dram_tensor("out", [b_loc, D], f32, kind="ExternalOutput")

    with tile.TileContext(nc) as tc, ExitStack() as ctx:
        const = ctx.enter_context(tc.tile_pool(name="const", bufs=1))
        g_T = const.tile([P, nd, s], f16)            # g_S.T resident
        whhT = const.tile([P, 2 * nd, 4 * H], f16)   # W_hh.T resident
        xw = const.tile([P, nb, 4 * H], f16)         # f_x@W_ih.T + biases
        fx32 = const.tile([P, nb, D], f32)
        br16 = const.tile([1, 4 * H], f16)
        ones16 = const.tile([1, P], f16)
        ident = const.tile([P, P], f16)

        dram = ctx.enter_context(tc.tile_pool(name="dram", bufs=1, space="DRAM"))
        g16d = dram.tile([s, D], f16)                # fp16 copy of g_S

        p_pool = ctx.enter_context(tc.tile_pool(name="p_pool", bufs=2))
        gsb_pool = ctx.enter_context(tc.tile_pool(name="gsb", bufs=3))
        pt_pool = ctx.enter_context(tc.tile_pool(name="ptp", bufs=3))
        ht_pool = ctx.enter_context(tc.tile_pool(name="htp", bufs=7))
        rt_pool = ctx.enter_context(tc.tile_pool(name="rtp", bufs=2))
        rh_pool = ctx.enter_context(tc.tile_pool(name="rhp", bufs=2))
        lstm_pool = ctx.enter_context(tc.tile_pool(name="lstm", bufs=2))
        z_pool = ctx.enter_context(tc.tile_pool(name="zp", bufs=4))
        st_pool = ctx.enter_context(tc.tile_pool(name="stp", bufs=2))

        ps_log = ctx.enter_context(tc.tile_pool(name="ps_log", bufs=3, space="PSUM"))
        ps_r = ctx.enter_context(tc.tile_pool(name="ps_r", bufs=1, space="PSUM"))
        ps_g = ctx.enter_context(tc.tile_pool(name="ps_g", bufs=2, space="PSUM"))
        ps_tp = ctx.enter_context(tc.tile_pool(name="ps_tp", bufs=2, space="PSUM"))

        make_identity(nc, ident[:])

        _tpn = [0]

        def tp_group(blocks, dst, copy_engine="v"):
            """PE-transpose len(blocks) [128,128] fp16 blocks into one fp16
            PSUM group tile, then one (possibly strided) copy into dst
            (shape [P, len(blocks), P])."""
            n = len(blocks)
            _tpn[0] += 1
            tp = ps_tp.tile([P, n, P], f16, tag="tp", name=f"tp_{_tpn[0]}")
            for t, blk in enumerate(blocks):
                nc.tensor.transpose(tp[:, t, :], blk, ident[:])
            if copy_engine == "v":
                nc.vector.tensor_copy(dst, tp[:])
            else:
                nc.scalar.copy(dst, tp[:])

        # ---------------- prolog ----------------
        nc.vector.memset(ones16[:], 1.0)

        bi16 = p_pool.tile([1, 4 * H], f16, tag="p", name="bi16")
        bh16 = p_pool.tile([1, 4 * H], f16, tag="p", name="bh16")
        nc.gpsimd.dma_start(bi16[:], b_ih[:].rearrange("(a n) -> a n", a=1))
        nc.gpsimd.dma_start(bh16[:], b_hh[:].rearrange("(a n) -> a n", a=1))
        nc.vector.scalar_tensor_tensor(br16[:], bi16[:], 0.0, bh16[:],
                                       op0=ALU.add, op1=ALU.add)

        # f_x: f32 copy + fp16 copy + transposed fp16 tiles
        fx16 = p_pool.tile([P, nb, D], f16, tag="p", name="fx16")
        for j in range(nb):
            nc.sync.dma_start(fx32[:, j, :], f_x[j * P:(j + 1) * P, :])
            nc.gpsimd.dma_start(fx16[:, j, :], f_x[j * P:(j + 1) * P, :])
        hT = {}
        for j in range(nb):
            t = ht_pool.tile([P, nd, P], f16, tag="hT", name=f"fxT_{j}")
            tp_group([fx16[:, j, kk * P:(kk + 1) * P] for kk in range(nd)], t[:])
            hT[j] = t

        # W_ih -> wihT (transposed fp16), then xw = f_x @ W_ih.T + biases
        wihT = p_pool.tile([P, nd, 4 * H], f16, tag="p", name="wihT")
        wtmp = p_pool.tile([P, (4 * H) // P, D], f16, tag="p", name="wtmp")
        for i in range((4 * H) // P):
            nc.gpsimd.dma_start(wtmp[:, i, :], W_ih[i * P:(i + 1) * P, :])
            tp_group([wtmp[:, i, kk * P:(kk + 1) * P] for kk in range(nd)],
                     wihT[:, :, i * P:(i + 1) * P])
        for j in range(nb):
            for n in range(ng):
                gp = ps_g.tile([P, 512], f32, tag="psg", name=f"xwps_{j}_{n}")
                nc.tensor.matmul(gp[:], ones16[:], br16[:, n * 512:(n + 1) * 512],
                                 start=True, stop=False)
                for kk in range(nd):
                    nc.tensor.matmul(gp[:], hT[j][:, kk, :],
                                     wihT[:, kk, n * 512:(n + 1) * 512],
                                     start=False, stop=(kk == nd - 1))
                nc.scalar.copy(xw[:, j, n * 512:(n + 1) * 512], gp[:])

        # W_hh -> whhT fp16 (transposed)
        for half in range(2):
            wh = p_pool.tile([P, (2 * H) // P, 2 * H], f16, tag="p",
                             name=f"wh_{half}")
            for i in range((2 * H) // P):
                nc.gpsimd.dma_start(
                    wh[:, i, :],
                    W_hh[half * 2 * H + i * P:half * 2 * H + (i + 1) * P, :])
                tp_group([wh[:, i, kk * P:(kk + 1) * P] for kk in range(2 * nd)],
                         whhT[:, :, half * 2 * H + i * P:half * 2 * H + (i + 1) * P])

        # g_S: fp16 scratch in DRAM + resident transposed copy g_T
        # (loaded in groups of 4 s-tiles per DMA to amortize descriptor cost)
        for tg4 in range(nt // 4):
            gt = gsb_pool.tile([P, 4, D], f16, tag="gsb", name=f"gload_{tg4}")
            nc.gpsimd.dma_start(
                gt[:], g_S[tg4 * 4 * P:(tg4 + 1) * 4 * P, :].rearrange(
                    "(a p) d -> p a d", p=P))
            nc.sync.dma_start(
                g16d[tg4 * 4 * P:(tg4 + 1) * 4 * P, :].rearrange(
                    "(a p) d -> p a d", p=P), gt[:])
            for a in range(4):
                t = tg4 * 4 + a
                tp_group([gt[:, a, kk * P:(kk + 1) * P] for kk in range(nd)],
                         g_T[:, :, t * P:(t + 1) * P],
                         copy_engine="v" if t % 2 == 0 else "s")

        # LSTM state: z = 2c = 0
        z = {}
        for j in range(nb):
            zt = z_pool.tile([P, D], f32, tag="z", name=f"z0_{j}")
            nc.vector.memset(zt[:], 0.0)
            z[j] = zt

        # ---------------- K steps ----------------
        pbuf, negmaxes, sums, fcorr, rsum = {}, {}, {}, {}, {}

        def emit_A(j):
            """logits + per-chunk negmax + exp for b-tile j"""
            pbuf[j] = p_pool.tile([P, s], f16, tag="p", name=f"p_{j}")
            negmaxes[j] = st_pool.tile([P, ns], f32, tag="nmx", name=f"nmx_{j}")
            sums[j] = st_pool.tile([P, ns], f32, tag="sums", name=f"sums_{j}")
            for i in range(ns):
                ps = ps_log.tile([P, 512], f32, tag="psl", name=f"psl_{j}_{i}")
                for kk in range(nd):
                    nc.tensor.matmul(
                        ps[:], hT[j][:, kk, :],
                        g_T[:, kk, i * 512:(i + 1) * 512],
                        start=(kk == 0), stop=(kk == nd - 1))
                nc.vector.tensor_reduce(
                    negmaxes[j][:, i:i + 1], ps[:],
                    axis=AX.X, op=ALU.max, negate=True)
                nc.scalar.activation(
                    pbuf[j][:, i * 512:(i + 1) * 512], ps[:],
                    AF.Exp, bias=negmaxes[j][:, i:i + 1],
                    accum_out=sums[j][:, i:i + 1])

        def emit_fin(j):
            """global max, correction factors, 1/sum for b-tile j"""
            nm = st_pool.tile([P, 1], f32, tag="nm", name=f"nm_{j}")
            nc.vector.tensor_reduce(nm[:], negmaxes[j][:], axis=AX.X, op=ALU.min)
            delta = st_pool.tile([P, ns], f32, tag="delta", name=f"delta_{j}")
            # delta_i = m_i - m = -negmax_i + nm
            nc.vector.tensor_scalar(delta[:], negmaxes[j][:], -1.0, nm[:],
                                    op0=ALU.mult, op1=ALU.add)
            fc = st_pool.tile([P, ns], f32, tag="fc", name=f"fc_{j}")
            nc.scalar.activation(fc[:], delta[:], AF.Exp)
            fcorr[j] = fc
            ws = st_pool.tile([P, ns], f32, tag="ws", name=f"ws_{j}")
            nc.vector.scalar_tensor_tensor(ws[:], sums[j][:], 0.0, fc[:],
                                           op0=ALU.add, op1=ALU.mult)
            ssum = st_pool.tile([P, 1], f32, tag="ssum", name=f"ssum_{j}")
            nc.vector.tensor_reduce(ssum[:], ws[:], axis=AX.X, op=ALU.add)
            rs = st_pool.tile([P, 1], f32, tag="rs", name=f"rs_{j}")
            nc.vector.reciprocal(rs[:], ssum[:])
            rsum[j] = rs

        def emit_B(j, k):
            """rescale p, transpose, readout, gates, LSTM update for b-tile j"""
            # p *= exp(m_i - m), in place; split between DVE and ACT
            for i in range(ns):
                sl = pbuf[j][:, i * 512:(i + 1) * 512]
                if i % 3 == 2:
                    nc.scalar.mul(sl, sl, fcorr[j][:, i:i + 1])
                else:
                    nc.vector.tensor_scalar_mul(sl, sl, fcorr[j][:, i:i + 1])
            # readout r = p~ @ g (accumulate over all s-tiles);
            # p transposed in groups of 4 via PE, 2 groups ahead of the mms
            rp = ps_r.tile([P, D], f32, tag="psr", name=f"psr_{j}")
            pTg = {}

            def tpg(ig):
                grp = pt_pool.tile([P, 4, P], f16, tag="pt", name=f"pt_{j}_{ig}")
                tp_group([pbuf[j][:, (ig * 4 + t) * P:(ig * 4 + t + 1) * P]
                          for t in range(4)], grp[:],
                         copy_engine="v" if ig % 2 == 0 else "s")
                pTg[ig] = grp

            gsbs = {}

            def gload(ig):
                gg = gsb_pool.tile([P, 4, D], f16, tag="gsb", name=f"gsb_{j}_{ig}")
                nc.sync.dma_start(
                    gg[:], g16d[ig * 4 * P:(ig + 1) * 4 * P, :].rearrange(
                        "(a p) d -> p a d", p=P))
                gsbs[ig] = gg

            tpg(0)
            tpg(1)
            gload(0)
            gload(1)
            for ig in range(nt // 4):
                if ig + 2 < nt // 4:
                    tpg(ig + 2)
                    gload(ig + 2)
                for t in range(4):
                    c = ig * 4 + t
                    nc.tensor.matmul(rp[:], pTg[ig][:, t, :], gsbs[ig][:, t, :],
                                     start=(c == 0), stop=(c == nt - 1))
                del pTg[ig]
                del gsbs[ig]
            r16 = rh_pool.tile([P, D], f16, tag="r16", name=f"r16_{j}")
            nc.vector.tensor_scalar_mul(r16[:], rp[:], rsum[j][:])
            rT = rt_pool.tile([P, nd, P], f16, tag="rT", name=f"rT_{j}")
            tp_group([r16[:, kk * P:(kk + 1) * P] for kk in range(nd)], rT[:])
            # gates = xw + h@Whh_h.T + r@Whh_r.T, tanh nonlinearities
            tt = [None] * ng
            for n in range(ng):
                gp = ps_g.tile([P, 512], f32, tag="psg", name=f"psg_{j}_{n}")
                for kk in range(nd):
                    nc.tensor.matmul(gp[:], hT[j][:, kk, :],
                                     whhT[:, kk, n * 512:(n + 1) * 512],
                                     start=(kk == 0), stop=False)
                for kk in range(nd):
                    nc.tensor.matmul(gp[:], rT[:, kk, :],
                                     whhT[:, nd + kk, n * 512:(n + 1) * 512],
                                     start=False, stop=(kk == nd - 1))
                pre = lstm_pool.tile([P, 512], f16, tag="pre", name=f"pre_{j}_{n}")
                nc.vector.scalar_tensor_tensor(
                    pre[:], gp[:], 0.0, xw[:, j, n * 512:(n + 1) * 512],
                    op0=ALU.add, op1=ALU.add)
                t = lstm_pool.tile([P, 512], f16, tag=f"t{n}", bufs=1,
                                   name=f"t{n}_{j}")
                # i,f,o gates: tanh(x/2) (-> sigmoid); g gate: tanh(x)
                nc.scalar.activation(t[:], pre[:], AF.Tanh,
                                     scale=1.0 if n == 2 else 0.5)
                tt[n] = t
            ti, tf, tg, to = tt
            # z' = 0.5*(tf+1)*z + (ti+1)*tg       (z = 2c)
            v = lstm_pool.tile([P, D], f16, tag="v", name=f"v_{j}")
            nc.vector.scalar_tensor_tensor(v[:], ti[:], 1.0, tg[:],
                                           op0=ALU.add, op1=ALU.mult)
            q = lstm_pool.tile([P, D], f16, tag="q", name=f"q_{j}")
            nc.vector.scalar_tensor_tensor(q[:], tf[:], 1.0, z[j][:],
                                           op0=ALU.add, op1=ALU.mult)
            zn = z_pool.tile([P, D], f32, tag="z", name=f"z_{j}")
            nc.vector.scalar_tensor_tensor(zn[:], q[:], 0.5, v[:],
                                           op0=ALU.mult, op1=ALU.add)
            z[j] = zn
            # h' = 0.5*(to+1)*tanh(z'/2) + f_x
            y = lstm_pool.tile([P, D], f16, tag="y", name=f"y_{j}")
            nc.scalar.activation(y[:], zn[:], AF.Tanh, scale=0.5)
            w = lstm_pool.tile([P, D], f16, tag="w", name=f"w_{j}")
            nc.vector.scalar_tensor_tensor(w[:], to[:], 1.0, y[:],
                                           op0=ALU.add, op1=ALU.mult)
            if k < k_steps - 1:
                h16 = rh_pool.tile([P, D], f16, tag="h16", name=f"h16_{j}")
                nc.vector.scalar_tensor_tensor(h16[:], w[:], 0.5, fx32[:, j, :],
                                               op0=ALU.mult, op1=ALU.add)
                hTn = ht_pool.tile([P, nd, P], f16, tag="hT", name=f"hT_{j}")
                tp_group([h16[:, kk * P:(kk + 1) * P] for kk in range(nd)], hTn[:])
                hT[j] = hTn
            else:
                ho = z_pool.tile([P, D], f32, tag="z", name=f"ho_{j}")
                nc.vector.scalar_tensor_tensor(ho[:], w[:], 0.5, fx32[:, j, :],
                                               op0=ALU.mult, op1=ALU.add)
                nc.sync.dma_start(out[j * P:(j + 1) * P, :], ho[:])

        for k in range(k_steps):
            # schedule: A(0) A(1) B(0) A(2) B(1) A(3) B(2) B(3)
            emit_A(0)
            emit_fin(0)
            if nb > 1:
                emit_A(1)
                emit_fin(1)
            emit_B(0, k)
            for j in range(2, nb):
                emit_A(j)
                emit_fin(j)
                emit_B(j - 1, k)
            if nb > 1:
                emit_B(nb - 1, k)

    return nc


_NC_CACHE = {}


def _get_nc():
    if "full" not in _NC_CACHE:
        nc = build_bass()
        nc.finalize()
        _NC_CACHE["full"] = nc
    return _NC_CACHE["full"]


def kernel(f_x, g_S, W_ih, W_hh, b_ih, b_hh):
    from concourse.bass_utils import run_bass_kernel_spmd

    nc = _get_nc()
    f_x = np.ascontiguousarray(f_x, dtype=np.float32)
    g_S = np.ascontiguousarray(g_S, dtype=np.float32)
    W_ih = np.ascontiguousarray(W_ih, dtype=np.float32)
    W_hh = np.ascontiguousarray(W_hh, dtype=np.float32)
    b_ih = np.ascontiguousarray(b_ih, dtype=np.float32)
    b_hh = np.ascontiguousarray(b_hh, dtype=np.float32)
    in_maps = [
        {
            "f_x": f_x[c * B_LOC:(c + 1) * B_LOC],
            "g_S": g_S,
            "W_ih": W_ih,
            "W_hh": W_hh,
            "b_ih": b_ih,
            "b_hh": b_hh,
        }
        for c in range(N_CORES)
    ]
    res = run_bass_kernel_spmd(nc, in_maps, core_ids=list(range(N_CORES)))
    return np.concatenate([res.results[c]["out"] for c in range(N_CORES)], axis=0)


if __name__ == "__main__":
    nc = build_bass()
    nc.finalize()
    print("built ok")
